# revision 48
# baseline (speedup 1.0000x reference)
"""Trainium2 Bass kernel for nn_CONVMGEmbedder (3-layer GraphConv + UnitedNorm + readout).

Strategy: dst-sharded graph partition over 8 NeuronCores.
- Node shard k = rows [k*12500, (k+1)*12500), padded to 12544 (98 blocks of 128).
- Table layout is CHUNK-major: 4 chunks of blocks (25/25/25/23) so each chunk is an
  independent AllGather output; per-chunk collectives overlap passB.
- Edges live on their dst-owner core, grouped by (dst block, src chunk), padded to a
  global (SPMD-uniform) tile table; pad slots gather row 0 and are nulled by their
  slot=-1 one-hot columns.
- Layer-0 table is host-prepped (nf * inv_sqrt_out, bf16) so S tiles are pure one-hot
  for every layer and there is no prologue scatter.
- Aggregation: per-segment dma_gather of m[src] rows, one-hot S tiles (DVE iota/
  is_equal), PE matmuls accumulate aggT = sum_e m_e (x) 1_slot.
- h = (aggT.T @ W) * inv_sqrt_in (ACT copy w/ per-node scale, fused row-sum for node
  stats); UnitedNorm via G-one-hot matmuls + 16KB AllReduce; per-node stats from
  fused ACT accumulations.
- m_{l+1} = leaky(out)*inv_sqrt_out, written in 5-block batches, AllGathered per
  chunk into the next layer's table.
- Readout: G^T @ h3 accumulated in PSUM, AllReduce, /cnt, leaky.
"""
import math
import os
import sys

sys.path.insert(0, "/opt/trn_rl_repo")

import numpy as np




def _cfg_real():
    return dict(
        N=100000, E=1600000, C=128, B=16, L=3, NCORES=8,
        GD="bf16",
    )


def _derive(cfg):
    c = dict(cfg)
    c["SHARD"] = c["N"] // c["NCORES"]
    # 100 blocks (vs the minimal 98) leaves ~2.3% slack per (core,block) bin so
    # the node packing can keep per-(core,block,chunk) in-edge counts <= 512
    c["NBLK"] = 100
    c["SHARD_PAD"] = c["NBLK"] * 128
    # chunk-major layout: 4 chunks of blocks
    c["CB"] = [25, 25, 25, 25]           # blocks per chunk
    c["R0B"] = [0, 25, 50, 75]           # first block of each chunk
    c["ROWS"] = [cb * 128 for cb in c["CB"]]   # rows per core per chunk
    c["NQ"] = 4
    for rj in c["ROWS"]:
        assert c["NCORES"] * rj <= 32768  # int16 gather index range
    c["EPS"] = 1e-5
    c["SLOPE"] = 0.2
    return c


def prep_host(inputs, cfg):
    """Pure-numpy sharding prep: degrees, edge reorder, tile tables, constants."""
    N, E, C, B = cfg["N"], cfg["E"], cfg["C"], cfg["B"]
    NC, NQ = cfg["NCORES"], cfg["NQ"]
    SHARD, NBLK = cfg["SHARD"], cfg["NBLK"]
    SHARD_PAD = cfg["SHARD_PAD"]
    CB, R0B, ROWS = cfg["CB"], cfg["R0B"], cfg["ROWS"]

    nf = np.asarray(inputs["node_feats"], np.float32)
    W = np.asarray(inputs["W"], np.float32)
    gamma = np.asarray(inputs["gamma"], np.float32)
    beta = np.asarray(inputs["beta"], np.float32)
    lam = np.asarray(inputs["lambdas"], np.float32)
    src = np.asarray(inputs["src"]).astype(np.int64)
    dst = np.asarray(inputs["dst"]).astype(np.int64)
    gid = np.asarray(inputs["graph_ids"]).astype(np.int64)

    deg_out = np.maximum(np.bincount(src, minlength=N).astype(np.float64), 1.0)
    deg_in = np.maximum(np.bincount(dst, minlength=N).astype(np.float64), 1.0)
    iso = (1.0 / np.sqrt(deg_out)).astype(np.float32)   # inv_sqrt_out per node
    isi = (1.0 / np.sqrt(deg_in)).astype(np.float32)    # inv_sqrt_in per node
    cnt = np.maximum(np.bincount(gid, minlength=B).astype(np.float64), 1.0)
    cnt_inv = (1.0 / cnt).astype(np.float32).reshape(B, 1)

    lam64 = lam.astype(np.float64)
    ex = np.exp(lam64 - lam64.max(axis=1, keepdims=True))
    wsoft = (ex / ex.sum(axis=1, keepdims=True)).astype(np.float64)  # [L,3]

    chunk_of_block = np.zeros(NBLK, np.int64)
    for j in range(NQ):
        chunk_of_block[R0B[j]:R0B[j] + CB[j]] = j

    # ---- node repacking ----
    # A node's (core, block, slot) is free to choose (the readout is per-graph).
    # Keep each node inside its original chunk so the chunk label of every
    # OUT-edge is preserved, then rebalance nodes across that chunk's
    # (core, block) bins so per-(core, block, src-chunk) in-edge counts stay
    # under 512 and most segments need 4 tiles instead of 5.
    # chunk membership: equal quartiles of each original shard, so every chunk
    # carries 25000 nodes against 25600 slots of capacity
    ochunk = (np.arange(N) % SHARD) // (SHARD // NQ)             # [N]
    indegq = np.bincount(dst * NQ + ochunk[src],
                         minlength=N * NQ).reshape(N, NQ)        # [N, NQ]
    core_of = np.empty(N, np.int64)
    blk_of = np.empty(N, np.int64)
    slot_of = np.empty(N, np.int64)
    perm = -np.ones((NC, SHARD_PAD), np.int64)   # (core, blk*128+slot) -> node
    for j in range(NQ):
        nodes_j = np.where(ochunk == j)[0]
        nbins = NC * CB[j]
        loads = np.zeros((nbins, NQ), np.int64)
        slots_left = np.full(nbins, 128, np.int64)
        vecs = indegq[nodes_j]
        order_j = np.argsort(-vecs.sum(axis=1), kind="stable")
        bin_of_local = np.empty(len(nodes_j), np.int64)
        for vi in order_j:
            v = nodes_j[vi]
            vec = vecs[vi]
            cand = loads + vec
            # only component overshoot beyond the 512-edge (4-tile) budget
            # matters; balance the rest as a tie-break
            over = np.maximum(cand - 512, 0).sum(axis=1)
            score = (over << 24) + cand.max(axis=1) * 256 - slots_left
            score[slots_left <= 0] = np.iinfo(np.int64).max
            bsel = int(np.argmin(score))
            loads[bsel] += vec
            slots_left[bsel] -= 1
            bin_of_local[vi] = bsel

        # swap refinement: pull the highest-degree-q node out of each overfull
        # (bin, q) into a bin with slack, swapping against a low-degree-q node
        members = [list(np.where(bin_of_local == bb)[0]) for bb in range(nbins)]
        for _ in range(6):
            changed = 0
            for qq in range(NQ):
                for bo in np.where(loads[:, qq] > 512)[0]:
                    mo = members[bo]
                    v1 = max(mo, key=lambda x: vecs[x, qq])
                    vec1 = vecs[v1]
                    done = False
                    for bu in np.argsort(loads[:, qq])[:32]:
                        if bu == bo:
                            continue
                        for v2 in sorted(members[bu],
                                         key=lambda x: vecs[x, qq])[:8]:
                            vec2 = vecs[v2]
                            nlo = loads[bo] - vec1 + vec2
                            nlu = loads[bu] - vec2 + vec1
                            oldov = (np.maximum(loads[bo] - 512, 0).sum()
                                     + np.maximum(loads[bu] - 512, 0).sum())
                            newov = (np.maximum(nlo - 512, 0).sum()
                                     + np.maximum(nlu - 512, 0).sum())
                            if newov < oldov:
                                loads[bo] = nlo
                                loads[bu] = nlu
                                members[bo].remove(v1)
                                members[bo].append(v2)
                                members[bu].remove(v2)
                                members[bu].append(v1)
                                bin_of_local[v1] = bu
                                bin_of_local[v2] = bo
                                changed += 1
                                done = True
                                break
                        if done:
                            break
            if changed == 0:
                break

        for bb in range(nbins):
            k = bb // CB[j]
            gb = R0B[j] + bb % CB[j]
            for sl, vi in enumerate(members[bb]):
                v = nodes_j[vi]
                core_of[v] = k
                blk_of[v] = gb
                slot_of[v] = sl
                perm[k, gb * 128 + sl] = v

    # edge -> (core, blk, slot) on dst side; (q, idx16) on src side (chunk-major)
    core = core_of[dst]
    blk = blk_of[dst]
    slot = slot_of[dst].astype(np.float32)
    q = ochunk[src]
    rows_q = np.array(ROWS, np.int64)[q]
    idx16 = (core_of[src] * rows_q
             + (blk_of[src] - np.array(R0B, np.int64)[q]) * 128
             + slot_of[src]).astype(np.int16)

    # counts per (core, blk, q)
    key = (core * NBLK + blk) * NQ + q
    cnts = np.bincount(key, minlength=NC * NBLK * NQ).reshape(NC, NBLK, NQ)
    T = np.ceil(cnts.max(axis=0) / 128.0).astype(np.int64)  # [NBLK, NQ]
    zero_blocks = T.sum(axis=1) == 0
    T[zero_blocks, 0] = 1

    TQ = T.sum(axis=0)          # tiles per bucket stream
    EQ = TQ * 128               # padded edge slots per stream
    off_blk = np.zeros((NBLK, NQ), np.int64)
    off_blk[1:] = np.cumsum(T[:-1] * 128, axis=0)

    order = np.lexsort((q, blk, core))   # sort edges by (core, blk, q)
    per_core = []
    for k in range(NC):
        sel = order[core[order] == k]
        bblk, bq = blk[sel], q[sel]
        grp = bblk * NQ + bq
        rank = np.zeros(len(sel), np.int64)
        if len(sel):
            gcnt = np.bincount(grp, minlength=NBLK * NQ)
            starts = np.concatenate([[0], np.cumsum(gcnt)[:-1]])
            rank = np.arange(len(sel)) - starts[grp]
        pos = off_blk[bblk, bq] + rank
        d = {}
        for qq in range(NQ):
            eq = int(EQ[qq])
            idx_q = np.zeros(eq, np.int16)   # pad slots gather row 0
            slot_q = -np.ones(eq, np.float32)
            m = bq == qq
            idx_q[pos[m]] = idx16[sel[m]]
            slot_q[pos[m]] = slot[sel[m]]
            d[f"idxq{qq}"] = np.tile(
                np.ascontiguousarray(idx_q.reshape(-1, 16).T), (8, 1))
            d[f"slotq{qq}"] = np.ascontiguousarray(slot_q.reshape(-1, 128).T)
        pk = perm[k]                     # [SHARD_PAD] node id or -1
        real = pk >= 0
        isi_k = np.ones(SHARD_PAD, np.float32)
        isi_k[real] = isi[pk[real]]
        iso_k = np.ones(SHARD_PAD, np.float32)
        iso_k[real] = iso[pk[real]]
        d["inv_in_c"] = np.ascontiguousarray(isi_k.reshape(NBLK, 128).T)
        d["inv_out_c"] = np.ascontiguousarray(iso_k.reshape(NBLK, 128).T)
        G = np.zeros((SHARD_PAD, B), np.float32)
        G[np.where(real)[0], gid[pk[real]]] = 1.0
        G3 = G.reshape(NBLK, 128, B)
        d["g_oh"] = np.ascontiguousarray(G3.transpose(1, 0, 2)).reshape(128, NBLK * B)
        d["g_ohT"] = np.ascontiguousarray(G3.transpose(2, 0, 1)).reshape(B, NBLK * 128)
        per_core.append(d)

    # layer-0 table, chunk-major, pre-scaled by inv_sqrt_out (replicated per core)
    nfs = nf * iso[:, None]
    m1c = []
    for j in range(NQ):
        t = np.zeros((NC * ROWS[j], C), np.float32)
        for k in range(NC):
            pkj = perm[k, R0B[j] * 128:R0B[j] * 128 + ROWS[j]]
            realj = pkj >= 0
            t[k * ROWS[j]:(k + 1) * ROWS[j]][realj] = nfs[pkj[realj]]
        m1c.append(t)

    consts = dict(
        iota=np.broadcast_to(np.arange(128, dtype=np.float32), (128, 128)).copy(),
        wmat=np.ascontiguousarray(W.transpose(1, 0, 2)).reshape(C, cfg["L"] * C),
        cnt_inv=cnt_inv,
        m1c=m1c,
        gamma=gamma, beta=beta,
    )
    gamma_trivial = bool(np.all(gamma == 1.0) and np.all(beta == 0.0))
    meta = dict(T=T, TQ=TQ, EQ=EQ, wsoft=wsoft, gamma_trivial=gamma_trivial,
                TMAX=int(T.max()))
    return meta, per_core, consts


def build_nc(cfg, meta):
    import concourse.bacc as bacc
    import concourse.bass as bass
    import concourse.mybir as mybir
    import concourse.tile as tile

    f32 = mybir.dt.float32
    GD = f32 if cfg["GD"] == "f32" else mybir.dt.bfloat16
    C, B, L = cfg["C"], cfg["B"], cfg["L"]
    NC, NQ = cfg["NCORES"], cfg["NQ"]
    NBLK, SHARD_PAD = cfg["NBLK"], cfg["SHARD_PAD"]
    CB, R0B, ROWS = cfg["CB"], cfg["R0B"], cfg["ROWS"]
    EPS, SLOPE, N = cfg["EPS"], cfg["SLOPE"], cfg["N"]
    T, TQ, EQ = meta["T"], meta["TQ"], meta["EQ"]
    wsoft, TMAX = meta["wsoft"], meta["TMAX"]
    gtriv = meta["gamma_trivial"]
    RG = [list(range(NC))]
    eq_ = mybir.AluOpType
    AF = mybir.ActivationFunctionType

    nc = bacc.Bacc("TRN2", target_bir_lowering=False, debug=False,
                   num_devices=NC, num_swdge_queues=NQ)

    # ---- DRAM tensors ----
    out_t = nc.dram_tensor("out", [B, C], f32, kind="ExternalOutput")
    idx_t, slot_t, m1c_t = [], [], []
    for q in range(NQ):
        idx_t.append(nc.dram_tensor(f"idxq{q}", [128, int(EQ[q]) // 16],
                                    mybir.dt.int16, kind="ExternalInput"))
        slot_t.append(nc.dram_tensor(f"slotq{q}", [128, int(EQ[q]) // 128],
                                     GD, kind="ExternalInput"))
        m1c_t.append(nc.dram_tensor(f"m1c{q}", [NC * ROWS[q], C], GD,
                                    kind="ExternalInput"))
    invin_t = nc.dram_tensor("inv_in_c", [128, NBLK], f32, kind="ExternalInput")
    invout_t = nc.dram_tensor("inv_out_c", [128, NBLK], f32, kind="ExternalInput")
    goh_t = nc.dram_tensor("g_oh", [128, NBLK * B], f32, kind="ExternalInput")
    gohT_t = nc.dram_tensor("g_ohT", [B, NBLK * 128], f32, kind="ExternalInput")
    iota_t = nc.dram_tensor("iota", [128, 128], GD, kind="ExternalInput")
    wmat_t = nc.dram_tensor("wmat", [C, L * C], f32, kind="ExternalInput")
    cntinv_t = nc.dram_tensor("cnt_inv", [B, 1], f32, kind="ExternalInput")
    gamma_t = nc.dram_tensor("gamma", [L, C], f32, kind="ExternalInput")
    beta_t = nc.dram_tensor("beta", [L, C], f32, kind="ExternalInput")

    # per-layer shard outputs and gathered tables, per chunk
    mshc, mfullc, stin, stout = [], [], [], []
    for l in range(L - 1):
        mshc.append([nc.dram_tensor(f"msh{l}_{j}", [ROWS[j], C], GD)
                     for j in range(NQ)])
        mfullc.append([nc.dram_tensor(f"mfull{l}_{j}", [NC * ROWS[j], C], GD,
                                      addr_space="Shared") for j in range(NQ)])
    for l in range(L):
        stin.append(nc.dram_tensor(f"stin{l}", [2 * B, C], f32))
        stout.append(nc.dram_tensor(f"stout{l}", [2 * B, C], f32,
                                    addr_space="Shared"))
    embin = nc.dram_tensor("embin", [B, C], f32)
    embout = nc.dram_tensor("embout", [B, C], f32, addr_space="Shared")

    # mt write batching: groups of blocks within each chunk
    WGRP = 5
    wgroups = []  # list of (chunk j, block start, nblocks, is_chunk_last)
    for j in range(NQ):
        b0 = R0B[j]
        nb = CB[j]
        for g0 in range(0, nb, WGRP):
            gn = min(WGRP, nb - g0)
            wgroups.append((j, b0 + g0, gn, g0 + gn == nb))
    grp_of_block = {}
    for gi, (j, bs, gn, last) in enumerate(wgroups):
        for bb in range(bs, bs + gn):
            grp_of_block[bb] = (gi, bb - bs, bb == bs + gn - 1, j, last)

    with tile.TileContext(nc) as tc:
        with (
            tc.tile_pool(name="const", bufs=1) as cp,
            tc.tile_pool(name="big", bufs=1) as bigp,
            tc.tile_pool(name="idxp", bufs=1) as ip,
            tc.tile_pool(name="gath", bufs=4) as gp,
            tc.tile_pool(name="work", bufs=3) as wp,
            tc.tile_pool(name="mtb", bufs=2) as mp,
            tc.tile_pool(name="coef", bufs=1) as kp,
            tc.tile_pool(name="psum", bufs=2, space="PSUM") as pp,
            tc.tile_pool(name="psum1", bufs=1, space="PSUM") as pp1,
        ):
            # ---- resident constants ----
            iota = cp.tile([128, 128], GD)
            nc.sync.dma_start(iota[:], iota_t.ap())
            wm = cp.tile([C, L, C], f32)
            nc.sync.dma_start(wm[:], wmat_t.ap().rearrange("c (l k) -> c l k", l=L))
            goh = cp.tile([128, NBLK, B], f32)
            nc.sync.dma_start(goh[:], goh_t.ap().rearrange("p (b g) -> p b g", b=NBLK))
            gohT = cp.tile([B, NBLK * 128], f32)
            nc.sync.dma_start(gohT[:], gohT_t.ap())
            invin = cp.tile([128, NBLK], f32)
            nc.sync.dma_start(invin[:], invin_t.ap())
            invout = cp.tile([128, NBLK], f32)
            nc.sync.dma_start(invout[:], invout_t.ap())
            cntinv = cp.tile([B, 1], f32)
            nc.sync.dma_start(cntinv[:], cntinv_t.ap())
            ones16 = cp.tile([B, 1], f32)
            nc.vector.memset(ones16[:], 1.0)
            ones1 = cp.tile([1, B], f32)
            nc.vector.memset(ones1[:], 1.0)
            ones1p = cp.tile([1, 128], f32)
            nc.vector.memset(ones1p[:], 1.0)
            eps128 = cp.tile([128, 1], f32)
            nc.vector.memset(eps128[:], EPS)
            gam = cp.tile([L, C], f32)
            nc.sync.dma_start(gam[:], gamma_t.ap())
            bet = cp.tile([L, C], f32)
            nc.sync.dma_start(bet[:], beta_t.ap())
            hbuf = bigp.tile([128, NBLK, C], f32)
            nm_arr = cp.tile([128, NBLK], f32)
            nv_arr = cp.tile([128, NBLK], f32)

            # idx/slot tables are layer-invariant: resident in SBUF for the
            # whole kernel
            idxs, slots = [], []
            for q in range(NQ):
                it = ip.tile([128, int(EQ[q]) // 16], mybir.dt.int16,
                             tag=f"i{q}", name=f"it{q}")
                nc.sync.dma_start(it[:], idx_t[q].ap())
                st = ip.tile([128, int(EQ[q]) // 128], GD,
                             tag=f"s{q}", name=f"st{q}")
                nc.sync.dma_start(st[:], slot_t[q].ap())
                idxs.append(it)
                slots.append(st)

            for l in range(L):
                tabs = [(m1c_t[j] if l == 0 else mfullc[l - 1][j]).ap()
                        for j in range(NQ)]
                w0, w1, w2 = [float(x) for x in wsoft[l]]

                # ---------------- PASS A ----------------
                scopeA = nc.named_scope(f"passA_{l}"); scopeA.__enter__()
                gs_p = pp1.tile([B, C], f32, tag="gs")
                gss_p = pp1.tile([B, C], f32, tag="gss")
                cur = [0] * NQ          # consumed tiles per stream

                for b in range(NBLK):
                    aggT_p = pp.tile([C, 128], f32, tag="aggT")
                    ntot = int(T[b].sum())
                    done = 0
                    for q in range(NQ):
                        nt = int(T[b, q])
                        if nt == 0:
                            continue
                        t0 = cur[q]
                        cur[q] += nt
                        gt = gp.tile([128, TMAX, C], GD, tag=f"g{q}")
                        nc.gpsimd.dma_gather(
                            gt[:, :nt, :], tabs[q],
                            idxs[q][:, t0 * 8:(t0 + nt) * 8],
                            nt * 128, nt * 128, C, queue_num=q)
                        S = gp.tile([128, TMAX, 128], GD, tag=f"S{q}", name=f"S{q}")
                        nc.vector.tensor_tensor(
                            out=S[:, :nt, :],
                            in0=iota[:].unsqueeze(1).broadcast_to([128, nt, 128]),
                            in1=slots[q][:, t0:t0 + nt].unsqueeze(2)
                                .broadcast_to([128, nt, 128]),
                            op=eq_.is_equal)
                        for j in range(nt):
                            nc.tensor.matmul(
                                aggT_p[:], gt[:, j, :], S[:, j, :],
                                start=(done == 0), stop=(done == ntot - 1))
                            done += 1
                    aggT_s = wp.tile([C, 128], f32, tag="aggTs")
                    nc.vector.tensor_copy(aggT_s[:], aggT_p[:])
                    h_p = pp.tile([128, C], f32, tag="hp")
                    nc.tensor.matmul(h_p[:], aggT_s[:], wm[:, l, :],
                                     start=True, stop=True)
                    nc.scalar.activation(hbuf[:, b, :], h_p[:], AF.Copy,
                                         scale=invin[:, b:b + 1],
                                         accum_out=nm_arr[:, b:b + 1])
                    h2 = wp.tile([128, C], f32, tag="h2")
                    nc.scalar.activation(h2[:], hbuf[:, b, :], AF.Square,
                                         accum_out=nv_arr[:, b:b + 1])
                    nc.tensor.matmul(gs_p[:], goh[:, b, :], hbuf[:, b, :],
                                     start=(b == 0), stop=(b == NBLK - 1))
                    nc.tensor.matmul(gss_p[:], goh[:, b, :], h2[:],
                                     start=(b == 0), stop=(b == NBLK - 1))

                scopeA.__exit__(None, None, None)
                scopeS = nc.named_scope(f"stats_{l}"); scopeS.__enter__()
                # ---- stats AllReduce ----
                sts_a = kp.tile([B, C], f32, tag="sts_a")
                nc.vector.tensor_copy(sts_a[:], gs_p[:])
                sts_b = kp.tile([B, C], f32, tag="sts_b")
                nc.vector.tensor_copy(sts_b[:], gss_p[:])
                nc.sync.dma_start(stin[l].ap()[0:B, :], sts_a[:])
                nc.sync.dma_start(stin[l].ap()[B:2 * B, :], sts_b[:])
                nc.gpsimd.collective_compute(
                    "AllReduce", eq_.add, ins=[stin[l].ap()],
                    outs=[stout[l].ap()], replica_groups=RG)
                gs_t = kp.tile([B, C], f32, tag="gs_t")
                nc.sync.dma_start(gs_t[:], stout[l].ap()[0:B, :])
                gss_t = kp.tile([B, C], f32, tag="gss_t")
                nc.sync.dma_start(gss_t[:], stout[l].ap()[B:2 * B, :])
                gs, gss = gs_t[:], gss_t[:]

                # ---- coefficients A16/B16 ----
                gm = kp.tile([B, C], f32, tag="gm")
                nc.vector.tensor_scalar_mul(gm[:], gs, cntinv[:])
                gv = kp.tile([B, C], f32, tag="gv")
                nc.vector.tensor_scalar_mul(gv[:], gss, cntinv[:])
                tmp16 = kp.tile([B, C], f32, tag="tmp16")
                nc.vector.tensor_tensor(out=tmp16[:], in0=gm[:], in1=gm[:], op=eq_.mult)
                nc.vector.tensor_tensor(out=gv[:], in0=gv[:], in1=tmp16[:], op=eq_.subtract)
                nc.scalar.activation(gv[:], gv[:], AF.Sqrt, bias=eps128[0:B, :])
                igv = kp.tile([B, C], f32, tag="igv")
                nc.vector.reciprocal(igv[:], gv[:])
                bs_p = pp.tile([1, C], f32, tag="aggT")
                nc.tensor.matmul(bs_p[:], ones16[:], gs, start=True, stop=True)
                bss_p = pp.tile([1, C], f32, tag="hp")
                nc.tensor.matmul(bss_p[:], ones16[:], gss, start=True, stop=True)
                bm = kp.tile([1, C], f32, tag="bm")
                nc.vector.tensor_scalar_mul(bm[:], bs_p[:], 1.0 / N)
                bv = kp.tile([1, C], f32, tag="bv")
                nc.vector.tensor_scalar_mul(bv[:], bss_p[:], 1.0 / N)
                tmp1 = kp.tile([1, C], f32, tag="tmp1")
                nc.vector.tensor_tensor(out=tmp1[:], in0=bm[:], in1=bm[:], op=eq_.mult)
                nc.vector.tensor_tensor(out=bv[:], in0=bv[:], in1=tmp1[:], op=eq_.subtract)
                nc.scalar.activation(bv[:], bv[:], AF.Sqrt, bias=eps128[0:1, :])
                ibv = kp.tile([1, C], f32, tag="ibv")
                nc.vector.reciprocal(ibv[:], bv[:])
                ibv_p = pp.tile([B, C], f32, tag="aggT")
                nc.tensor.matmul(ibv_p[:], ones1[:], ibv[:], start=True, stop=True)
                bmibv = kp.tile([1, C], f32, tag="bmibv")
                nc.vector.tensor_tensor(out=bmibv[:], in0=bm[:], in1=ibv[:], op=eq_.mult)
                bmibv_p = pp.tile([B, C], f32, tag="hp")
                nc.tensor.matmul(bmibv_p[:], ones1[:], bmibv[:], start=True, stop=True)
                A16 = kp.tile([B, C], f32, tag="A16")
                nc.vector.tensor_scalar_mul(A16[:], igv[:], w1)
                t16b = kp.tile([B, C], f32, tag="t16b")
                nc.vector.tensor_scalar_mul(t16b[:], ibv_p[:], w0)
                nc.vector.tensor_tensor(out=A16[:], in0=A16[:], in1=t16b[:], op=eq_.add)
                B16 = kp.tile([B, C], f32, tag="B16")
                nc.vector.tensor_tensor(out=B16[:], in0=gm[:], in1=igv[:], op=eq_.mult)
                nc.vector.tensor_scalar_mul(B16[:], B16[:], w1)
                nc.vector.tensor_scalar_mul(t16b[:], bmibv_p[:], w0)
                nc.vector.tensor_tensor(out=B16[:], in0=B16[:], in1=t16b[:], op=eq_.add)

                # ---- per-node coefficients ----
                nmm = kp.tile([128, NBLK], f32, tag="nmm")
                nc.vector.tensor_scalar_mul(nmm[:], nm_arr[:], 1.0 / C)
                nvm = kp.tile([128, NBLK], f32, tag="nvm")
                nc.vector.tensor_scalar_mul(nvm[:], nv_arr[:], 1.0 / C)
                nm2 = kp.tile([128, NBLK], f32, tag="nm2")
                nc.vector.tensor_tensor(out=nm2[:], in0=nmm[:], in1=nmm[:], op=eq_.mult)
                nc.vector.tensor_tensor(out=nvm[:], in0=nvm[:], in1=nm2[:], op=eq_.subtract)
                nc.scalar.activation(nvm[:], nvm[:], AF.Sqrt, bias=eps128[:])
                invn = kp.tile([128, NBLK], f32, tag="invn")
                nc.vector.reciprocal(invn[:], nvm[:])
                a_n = kp.tile([128, NBLK], f32, tag="a_n")
                nc.vector.tensor_scalar_mul(a_n[:], invn[:], w2)
                b_n = kp.tile([128, NBLK], f32, tag="b_n")
                nc.vector.tensor_tensor(out=b_n[:], in0=nmm[:], in1=a_n[:], op=eq_.mult)

                scopeS.__exit__(None, None, None)
                if l == L - 1:
                    emb_p = pp1.tile([B, C], f32, tag="emb")

                # ---------------- PASS B ----------------
                scopeB = nc.named_scope(f"passB_{l}"); scopeB.__enter__()
                mtb = None
                for b in range(NBLK):
                    A_p = pp.tile([128, C], f32, tag="aggT")
                    nc.tensor.matmul(A_p[:], gohT[:, b * 128:(b + 1) * 128], A16[:],
                                     start=True, stop=True)
                    B_p = pp.tile([128, C], f32, tag="hp")
                    nc.tensor.matmul(B_p[:], gohT[:, b * 128:(b + 1) * 128], B16[:],
                                     start=True, stop=True)
                    h = hbuf[:, b, :]
                    u = wp.tile([128, C], f32, tag="u")
                    nc.vector.tensor_scalar(u[:], h, a_n[:, b:b + 1],
                                            b_n[:, b:b + 1], eq_.mult, eq_.subtract)
                    v = wp.tile([128, C], f32, tag="v")
                    nc.vector.tensor_tensor(out=v[:], in0=h, in1=A_p[:], op=eq_.mult)
                    nc.vector.tensor_tensor(out=u[:], in0=u[:], in1=v[:], op=eq_.add)
                    nc.vector.tensor_tensor(out=u[:], in0=u[:], in1=B_p[:], op=eq_.subtract)
                    if not gtriv:
                        gam_p = pp.tile([128, C], f32, tag="aggT")
                        nc.tensor.matmul(gam_p[:], ones1p[:], gam[l:l + 1, :],
                                         start=True, stop=True)
                        bet_p = pp.tile([128, C], f32, tag="hp")
                        nc.tensor.matmul(bet_p[:], ones1p[:], bet[l:l + 1, :],
                                         start=True, stop=True)
                        nc.vector.tensor_tensor(out=u[:], in0=u[:], in1=gam_p[:], op=eq_.mult)
                        nc.vector.tensor_tensor(out=u[:], in0=u[:], in1=bet_p[:], op=eq_.add)
                    nc.vector.tensor_scalar_mul(v[:], u[:], SLOPE)
                    nc.vector.tensor_tensor(out=u[:], in0=u[:], in1=v[:], op=eq_.max)
                    if l < L - 1:
                        gi, off_in_grp, grp_end, j, chunk_end = grp_of_block[b]
                        if off_in_grp == 0:
                            mtb = mp.tile([128, WGRP, C], GD, tag="mtb")
                        nc.vector.tensor_scalar_mul(mtb[:, off_in_grp, :], u[:],
                                                    invout[:, b:b + 1])
                        if grp_end:
                            _, bs, gn, _ = wgroups[gi]
                            r0 = (bs - R0B[j]) * 128
                            nc.sync.dma_start(
                                mshc[l][j].ap()[r0:r0 + gn * 128, :]
                                .rearrange("(g p) c -> p g c", g=gn),
                                mtb[:, :gn, :])
                            if chunk_end:
                                with nc.named_scope(f"ag_{l}_{j}"):
                                    nc.gpsimd.collective_compute(
                                        "AllGather", eq_.bypass,
                                        ins=[mshc[l][j].ap()],
                                        outs=[mfullc[l][j].ap()],
                                        replica_groups=RG)
                    else:
                        nc.tensor.matmul(emb_p[:], goh[:, b, :], u[:],
                                         start=(b == 0), stop=(b == NBLK - 1))
                scopeB.__exit__(None, None, None)

            # ---- readout ----
            embs = kp.tile([B, C], f32, tag="embs")
            nc.vector.tensor_copy(embs[:], emb_p[:])
            nc.sync.dma_start(embin.ap(), embs[:])
            nc.gpsimd.collective_compute(
                "AllReduce", eq_.add, ins=[embin.ap()], outs=[embout.ap()],
                replica_groups=RG)
            embg = kp.tile([B, C], f32, tag="embg")
            nc.sync.dma_start(embg[:], embout.ap())
            nc.vector.tensor_scalar_mul(embg[:], embg[:], cntinv[:])
            embg2 = kp.tile([B, C], f32, tag="embg2")
            nc.vector.tensor_scalar_mul(embg2[:], embg[:], SLOPE)
            nc.vector.tensor_tensor(out=embg[:], in0=embg[:], in1=embg2[:], op=eq_.max)
            nc.sync.dma_start(out_t.ap(), embg[:])

    nc.compile()
    return nc


def make_in_maps(cfg, per_core, consts):
    import ml_dtypes
    GD_np = np.float32 if cfg["GD"] == "f32" else ml_dtypes.bfloat16
    base = dict(
        iota=consts["iota"].astype(GD_np), wmat=consts["wmat"],
        cnt_inv=consts["cnt_inv"], gamma=consts["gamma"], beta=consts["beta"],
    )
    for j, t in enumerate(consts["m1c"]):
        base[f"m1c{j}"] = t.astype(GD_np)
    in_maps = []
    for k in range(cfg["NCORES"]):
        d = dict(base)
        for kk, vv in per_core[k].items():
            if kk.startswith("slotq"):
                vv = vv.astype(GD_np)
            d[kk] = vv
        in_maps.append(d)
    return in_maps


_BUILD_CACHE = {}


def _build_cached(cfg, meta):
    key = (tuple(sorted((k, str(v)) for k, v in cfg.items())),
           meta["T"].tobytes(), meta["wsoft"].tobytes(), meta["gamma_trivial"])
    if key not in _BUILD_CACHE:
        _BUILD_CACHE[key] = build_nc(cfg, meta)
    return _BUILD_CACHE[key]


def run_cfg(cfg, inputs, trace=False, verbose=False):
    import time
    from concourse.bass_utils import run_bass_kernel_spmd
    t0 = time.time()
    cfg = _derive(cfg)
    meta, per_core, consts = prep_host(inputs, cfg)
    t1 = time.time()
    nc = _build_cached(cfg, meta)
    t2 = time.time()
    in_maps = make_in_maps(cfg, per_core, consts)
    res = run_bass_kernel_spmd(nc, in_maps, list(range(cfg["NCORES"])),
                               trace=trace)
    t3 = time.time()
    if verbose:
        print(f"[timing] prep={t1-t0:.1f}s build+compile={t2-t1:.1f}s "
              f"run={t3-t2:.1f}s", flush=True)
    return res.results[0]["out"].astype(np.float32), res


def kernel(**inputs):
    out, _ = run_cfg(_cfg_real(), inputs)
    return out
